# revision 1
# baseline (speedup 1.0000x reference)
"""CSWin self-attention Trainium2 kernel (v4).

Sharding: data-parallel over batch B=8 across 8 cores (1 image per core).
Per-core pipeline (image = 128x128 spatial, C=256):
  A) x loaded HBM->SBUF bf16 via SWDGE cast-DMA in 8 chunks (16 token-tiles
     each). LN stats via bn_stats/bn_aggr; rstd via Quake-style rsqrt on DVE
     (int bit-trick + 2 Newton steps) so ScalarE's activation table stays
     pinned to the exp set. Normalize on DVE (bf16 4x mode), then xbar
     DMA-transpose halves to channel-major y^T on the sync/scalar queues.
  B) Per direction (H interleaved with phase-A chunks for overlap, then V),
     per stripe (64 stripes of 2 rows/cols = seq 256, 4 heads x hd 32):
     qkv matmuls (PSUM groups kept sequential per bank), S^T row-tiled
     4 heads (K=32) into two double-buffered [128,1024] psum halves, exp per
     half on ScalarE, attn@V col-tiled per head (M=32 at 4 positions) so O
     lands channel-aligned in psum, softmax denominators via ones-lhsT
     matmuls into a second aligned psum, then reciprocal_approx_fast + mul
     straight into hHt/hVt. No SBUF<->SBUF compaction DMAs at all.
  C) Projection in groups of 8 token-tiles: matmuls into a [128,2048] psum,
     x group prefetched fp32 via SWDGE, residual add on DVE, one batched
     store per group.
"""

import numpy as np
import ml_dtypes

import concourse.bass as bass
import concourse.bacc as bacc
import concourse.mybir as mybir
import concourse.tile as tile
from concourse.bass_utils import run_bass_kernel_spmd

F32 = mybir.dt.float32
BF16 = mybir.dt.bfloat16
I32 = mybir.dt.int32
AF = mybir.ActivationFunctionType
ALU = mybir.AluOpType

B = 8
HH = 128
WW = 128
C = 256
T = HH * WW          # 16384 tokens
NT = T // 128        # 128 token tiles
NCHUNK = 8           # phase A chunks
TPC = NT // NCHUNK   # 16 tiles per chunk
NS = 64              # stripes per direction
SEQ = 256
NHD = 4
HD = 32
SCALE = HD ** -0.5
EPS = 1e-5


def build_nc(has_qbias: bool, has_pbias: bool) -> bass.Bass:
    nc = bacc.Bacc("TRN2", target_bir_lowering=False, debug=False)
    x_h = nc.dram_tensor("x", [T, C], F32, kind="ExternalInput")
    wqkv_h = nc.dram_tensor("wqkv", [2, 128, 768], BF16, kind="ExternalInput")
    wproj_h = nc.dram_tensor("wproj", [2, 128, 256], BF16, kind="ExternalInput")
    bqkv_h = nc.dram_tensor("bqkv", [1, 768], BF16, kind="ExternalInput")
    bproj_h = nc.dram_tensor("bproj", [1, 256], BF16, kind="ExternalInput")
    out_h = nc.dram_tensor("out", [T, C], F32, kind="ExternalOutput")

    with tile.TileContext(nc) as tc, tc.tile_pool(name="persist", bufs=1) as pp:
        # ---------------- persistent SBUF ----------------
        ytA = pp.tile([128, T], BF16, name="ytA", tag="ytA")
        ytB = pp.tile([128, T], BF16, name="ytB", tag="ytB")
        hHt = pp.tile([128, T], BF16, name="hHt", tag="hHt")
        hVt = pp.tile([128, T], BF16, name="hVt", tag="hVt")
        wqkv = pp.tile([128, 2 * 768], BF16, name="wqkv", tag="wqkv")
        wproj = pp.tile([128, 2 * 256], BF16, name="wproj", tag="wproj")
        brow = pp.tile([1, 768], BF16, name="brow", tag="brow")
        bprow = pp.tile([1, 256], BF16, name="bprow", tag="bprow")
        ones = pp.tile([1, 256], BF16, name="ones", tag="ones")
        ones32 = pp.tile([128, 32], BF16, name="ones32", tag="ones32")
        # double-buffered v tiles
        vts = [pp.tile([128, 2, 4, 32], BF16, name=f"vt{i}", tag=f"vt{i}")
               for i in range(2)]

        nc.sync.dma_start(out=wqkv[:, 0:768], in_=wqkv_h[0])
        nc.sync.dma_start(out=wqkv[:, 768:1536], in_=wqkv_h[1])
        nc.sync.dma_start(out=wproj[:, 0:256], in_=wproj_h[0])
        nc.sync.dma_start(out=wproj[:, 256:512], in_=wproj_h[1])
        if has_qbias:
            nc.sync.dma_start(out=brow[:], in_=bqkv_h[:])
        if has_pbias:
            nc.sync.dma_start(out=bprow[:], in_=bproj_h[:])
        nc.vector.memset(ones[:], 1.0)
        nc.vector.memset(ones32[:], 1.0)

        yview = [None, None]  # set after phase A pools open

        def stripe(di, g, qk_pool, v_idx, s_pool, o_pool, qksb_pool, esb_pool,
                   drec_pool):
            horiz = di == 0
            qoff = 0 if horiz else 128
            hdst = hHt if horiz else hVt
            # [128, 2, 128]: H rows (h, w-inner); V cols (w, h-inner)
            rv = [yv[:, 2 * g:2 * g + 2, :] for yv in yview[di]]
            # ---- qkv (groups sequential per PSUM bank) ----
            qk_ps = qk_pool.tile([128, 512], F32, tag="qkps")
            v_ps = qk_pool.tile([128, 256], F32, tag="vps")
            for qk in range(2):  # 0 = q, 1 = k
                col = qk * 256
                woff = qoff + qk * 256
                for kc in range(2):
                    nc.tensor.matmul(
                        qk_ps[:, col:col + 256],
                        lhsT=wqkv[:, kc * 768 + woff:kc * 768 + woff + 128],
                        rhs=rv[kc], start=kc == 0,
                        stop=kc == 1 and not has_qbias)
                if has_qbias:
                    nc.tensor.matmul(
                        qk_ps[:, col:col + 256], lhsT=brow[:, woff:woff + 128],
                        rhs=ones[:, 0:256], start=False, stop=True)
            for sc in range(2):
                for kc in range(2):
                    nc.tensor.matmul(
                        v_ps[:, sc * 128:sc * 128 + 128],
                        lhsT=rv[kc][:, sc, :],
                        rhs=wqkv[:, kc * 768 + 512 + qoff:kc * 768 + 640 + qoff],
                        start=kc == 0, stop=kc == 1 and not has_qbias)
                if has_qbias:
                    nc.tensor.matmul(
                        v_ps[:, sc * 128:sc * 128 + 128],
                        lhsT=ones[:, 0:128],
                        rhs=brow[:, 512 + qoff:640 + qoff],
                        start=False, stop=True)
            qk_sb = qksb_pool.tile([128, 512], BF16, tag="qksb")
            nc.vector.tensor_copy(qk_sb[:], qk_ps[:])
            vt = vts[v_idx]
            nc.vector.tensor_copy(
                vt[:], v_ps[:].rearrange("p (s h d) -> p s h d", s=2, h=4))
            # ---- S^T (row-tiled 4 heads, K=32), two psum halves ----
            e_sb = esb_pool.tile([128, 2048], BF16, tag="esb")
            for half in range(2):
                s_ps = s_pool.tile([128, 1024], F32, tag="sps")
                for hh in range(2):
                    h = 2 * half + hh
                    for sc in range(2):
                        nc.tensor.matmul(
                            s_ps[:, hh * 512 + sc * 256:hh * 512 + sc * 256 + 256],
                            lhsT=qk_sb[32 * h:32 * h + 32,
                                       256 + sc * 128:384 + sc * 128],
                            rhs=qk_sb[32 * h:32 * h + 32, 0:256],
                            start=True, stop=True,
                            tile_position=(32 * h, 0))
                nc.scalar.activation(
                    e_sb[:, half * 1024:(half + 1) * 1024], s_ps[:],
                    AF.Exp, scale=SCALE)
            # ---- attn @ V col-tiled per head (M=32) + denominators ----
            o_ps = o_pool.tile([128, 256], F32, tag="ops")
            d_ps = o_pool.tile([128, 256], F32, tag="dps")
            for sc in range(2):
                for h in range(NHD):
                    esl = e_sb[:, h * 512 + sc * 256:h * 512 + sc * 256 + 256]
                    nc.tensor.matmul(
                        o_ps[32 * h:32 * h + 32, :],
                        lhsT=vt[:, sc, h, :], rhs=esl,
                        start=sc == 0, stop=sc == 1,
                        tile_position=(0, 32 * h), skip_group_check=True)
                    nc.tensor.matmul(
                        d_ps[32 * h:32 * h + 32, :],
                        lhsT=ones32[:], rhs=esl,
                        start=sc == 0, stop=sc == 1,
                        tile_position=(0, 32 * h), skip_group_check=True)
            # ---- normalize straight into h^T ----
            drec = drec_pool.tile([128, 256], F32, tag="drec")
            nc.vector.reciprocal_approx_fast(drec[:], d_ps[:])
            nc.vector.tensor_mul(
                hdst[:, g * 256:(g + 1) * 256], o_ps[:], drec[:])

        # ---------------- phases A+B ----------------
        with (
            tc.tile_pool(name="xch", bufs=2) as xch_pool,
            tc.tile_pool(name="stat", bufs=2) as stat_pool,
            tc.tile_pool(name="ynrm", bufs=3) as ynrm_pool,
            tc.tile_pool(name="qkps", bufs=1, space="PSUM") as qk_pool,
            tc.tile_pool(name="sps", bufs=2, space="PSUM") as s_pool,
            tc.tile_pool(name="ops", bufs=1, space="PSUM") as o_pool,
            tc.tile_pool(name="qksb", bufs=3) as qksb_pool,
            tc.tile_pool(name="esb", bufs=2) as esb_pool,
            tc.tile_pool(name="drec", bufs=2) as drec_pool,
        ):
            yview = [
                [ytA[:].rearrange("p (h w) -> p h w", h=HH),
                 ytB[:].rearrange("p (h w) -> p h w", h=HH)],
                [ytA[:].rearrange("p (h w) -> p w h", h=HH),
                 ytB[:].rearrange("p (h w) -> p w h", h=HH)],
            ]
            for ch in range(NCHUNK):
                # -- phase A chunk: load, stats, rstd, normalize, transpose --
                xch = xch_pool.tile([128, TPC, 256], BF16, tag="xch")
                nc.gpsimd.dma_start(
                    out=xch[:],
                    in_=x_h[ch * TPC * 128:(ch + 1) * TPC * 128, :]
                    .rearrange("(t p) c -> p t c", t=TPC))
                st = stat_pool.tile([128, TPC, 6], F32, tag="st")
                mv = stat_pool.tile([128, TPC, 2], F32, tag="mv")
                for t in range(TPC):
                    nc.vector.bn_stats(st[:, t, :], xch[:, t, :])
                    nc.vector.bn_aggr(mv[:, t, :], st[:, t, :])
                # rstd = (var+eps)^-1/2 on DVE: Quake seed + 2 NR iterations
                v1 = stat_pool.tile([128, TPC], F32, tag="v1")
                r0 = stat_pool.tile([128, TPC], F32, tag="r0")
                aa = stat_pool.tile([128, TPC], F32, tag="aa")
                uu = stat_pool.tile([128, TPC], F32, tag="uu")
                r1 = stat_pool.tile([128, TPC], F32, tag="r1")
                rstd = stat_pool.tile([128, TPC], F32, tag="rstd")
                nc.vector.tensor_scalar_add(v1[:], mv[:, :, 1], EPS)
                nc.vector.tensor_scalar(
                    aa[:].bitcast(I32), v1[:].bitcast(I32), 1, None,
                    ALU.logical_shift_right)
                nc.vector.tensor_scalar(
                    r0[:].bitcast(I32), aa[:].bitcast(I32), -1, 0x5F3759DF,
                    ALU.mult, ALU.add)
                for rin, rout in ((r0, r1), (r1, rstd)):
                    nc.vector.tensor_mul(aa[:], rin[:], rin[:])
                    nc.vector.tensor_mul(uu[:], aa[:], v1[:])
                    nc.vector.tensor_scalar(
                        uu[:], uu[:], -0.5, 1.5, ALU.mult, ALU.add)
                    nc.vector.tensor_mul(rout[:], rin[:], uu[:])
                for t in range(TPC):
                    i = ch * TPC + t
                    y = ynrm_pool.tile([128, 256], BF16, tag="y")
                    nc.vector.tensor_scalar(
                        y[:], xch[:, t, :], mv[:, t, 0:1], rstd[:, t:t + 1],
                        ALU.subtract, ALU.mult)
                    nc.sync.dma_start(
                        out=ytA[:, i * 128:(i + 1) * 128], in_=y[:, 0:128],
                        transpose=True)
                    nc.scalar.dma_start(
                        out=ytB[:, i * 128:(i + 1) * 128], in_=y[:, 128:256],
                        transpose=True)
                # -- H stripes for this chunk --
                for k in range(NCHUNK):
                    g = ch * NCHUNK + k
                    stripe(0, g, qk_pool, g % 2, s_pool, o_pool, qksb_pool,
                           esb_pool, drec_pool)
            # -- V stripes --
            for g in range(NS):
                stripe(1, g, qk_pool, g % 2, s_pool, o_pool, qksb_pool,
                       esb_pool, drec_pool)

        # ---------------- phase C: projection + residual ----------------
        GT = 8                     # token tiles per group
        NG = NT // GT              # 16 groups
        hVv = hVt[:].rearrange("p (w h) -> p h w", h=HH)
        with (
            tc.tile_pool(name="pps", bufs=2, space="PSUM") as p_pool,
            tc.tile_pool(name="po", bufs=3) as po_pool,
            tc.tile_pool(name="xres", bufs=2) as xres_pool,
        ):
            for gi in range(NG):
                xres = xres_pool.tile([128, GT, 256], F32, tag="xres")
                nc.gpsimd.dma_start(
                    out=xres[:],
                    in_=x_h[gi * GT * 128:(gi + 1) * GT * 128, :]
                    .rearrange("(t p) c -> p t c", t=GT))
                p_ps = p_pool.tile([128, GT * 256], F32, tag="pps")
                for t in range(GT):
                    i = gi * GT + t
                    nc.tensor.matmul(
                        p_ps[:, t * 256:(t + 1) * 256],
                        lhsT=hHt[:, i * 128:(i + 1) * 128],
                        rhs=wproj[:, 0:256], start=True, stop=False)
                    nc.tensor.matmul(
                        p_ps[:, t * 256:(t + 1) * 256],
                        lhsT=hVv[:, i, :],
                        rhs=wproj[:, 256:512], start=False, stop=not has_pbias)
                    if has_pbias:
                        nc.tensor.matmul(
                            p_ps[:, t * 256:(t + 1) * 256],
                            lhsT=ones[:, 0:128], rhs=bprow[:],
                            start=False, stop=True)
                po = po_pool.tile([128, GT * 256], F32, tag="po")
                nc.vector.tensor_add(
                    po[:], p_ps[:], xres[:].rearrange("p t c -> p (t c)"))
                og = (out_h[gi * GT * 128:(gi + 1) * GT * 128, :]
                      .rearrange("(t p) c -> p t c", t=GT))
                nc.sync.dma_start(out=og,
                                  in_=po[:].rearrange("p (t c) -> p t c", t=GT))

    return nc


_NC_CACHE = {}


def _get_nc(has_qbias, has_pbias):
    key = (has_qbias, has_pbias)
    if key not in _NC_CACHE:
        nc = build_nc(has_qbias, has_pbias)
        nc.finalize()
        _NC_CACHE[key] = nc
    return _NC_CACHE[key]


def kernel(x, Wqkv, bqkv, Wproj, bproj, gamma, beta, _trace=False):
    x = np.asarray(x, np.float32)
    Wqkv = np.asarray(Wqkv, np.float32)
    bqkv = np.asarray(bqkv, np.float32)
    Wproj = np.asarray(Wproj, np.float32)
    bproj = np.asarray(bproj, np.float32)
    gamma = np.asarray(gamma, np.float32)
    beta = np.asarray(beta, np.float32)

    Wg = gamma[:, None] * Wqkv                      # fold LN affine scale
    bq = beta @ Wqkv + bqkv                         # fold LN affine shift
    has_qbias = bool(np.any(bq != 0.0))
    has_pbias = bool(np.any(bproj != 0.0))

    bf = ml_dtypes.bfloat16
    wqkv_np = np.ascontiguousarray(Wg.reshape(2, 128, 768)).astype(bf)
    wproj_np = np.ascontiguousarray(Wproj.reshape(2, 128, 256)).astype(bf)
    bq_np = bq.reshape(1, 768).astype(bf)
    bp_np = bproj.reshape(1, 256).astype(bf)

    nc = _get_nc(has_qbias, has_pbias)
    in_maps = []
    for b in range(B):
        in_maps.append({
            "x": np.ascontiguousarray(x[b].reshape(T, C)),
            "wqkv": wqkv_np, "wproj": wproj_np,
            "bqkv": bq_np, "bproj": bp_np,
        })
    res = run_bass_kernel_spmd(nc, in_maps, list(range(B)), trace=_trace)
    out = np.stack([np.asarray(res.results[b]["out"]).reshape(HH, WW, C)
                    for b in range(B)])
    if _trace:
        return out.astype(np.float32), res
    return out.astype(np.float32)

